# revision 8
# baseline (speedup 1.0000x reference)
"""Trainium2 Bass kernel for CompositionModel (gnn_message_passing).

Model: per-cell MLP over [log1p(X) ++ Z[cell_to_batch]] followed by a
segment-mean over batch labels.

v2 strategy (all-fp8 DoubleRow):
  * Host: log1p(X) computed on host, shipped as fp8. Cells sorted by segment,
    padded to 64-cell minichunks; minichunks assigned to even/odd blocks with
    per-segment parity balance (so alternate-block corrections average out
    per segment). Moving tile per 512-cell block is [128, 2, 512] fp8:
      ktile0       = Q8(log1p(X))                    (128 rows)
      ktile1[0:32] = Q8(Zc)            (hi)
      ktile1[32:64]= Q8(16*(Zc-hi))    (lo, exact-ish Z)
      ktile1[64]   = 1.0  -> stationary carries Q8(S1*b1)       (bias hi)
      ktile1[65]   = 1.0  -> stationary carries Q8(residual b1) (bias lo)
      ktile1[66:128] = dup of X rows [0:62] (even blk) / [62:124] (odd blk)
        -> stationary carries 2*(S1*W1x - Q8(S1*W1x)) : the W1 quantization
           error correction applied to half the rows on alternate blocks
           (first-order exact through the segment mean).
  * Device per block: L1 = 2 DR matmuls (K=256 incl. Z+bias+W1lo packed) ->
    ps1 [128,1024] f32 (2 banks) -> single DVE max(x,0) -> fp8 h1 ->
    L2 = 2 DR matmuls hi (+2 lo on even blocks) -> ps2 [128,1024] ->
    2 ACT relu+bias(b2 half) -> bf16 h2 -> GpSimd fold (64->32) ->
    DVE fold (32->16) -> DVE grouped tensor_reduce -> per-minichunk sums.
  * W3/b3 applied on host to the 512x256 segment sums; pad-cell contribution
    subtracted analytically (parity-dependent).
"""

import numpy as np
import ml_dtypes

import concourse.bacc as bacc
import concourse.mybir as mybir
import concourse.tile as tile
from concourse.bass_utils import run_bass_kernel_spmd

BF16 = ml_dtypes.bfloat16
FP8 = ml_dtypes.float8_e4m3fn

N_CORES = 8
DX = 128
DZ = 32
H = 256
B = 512
MC = 64            # minichunk: cells per single-segment group
BLK = 512          # cells per device block
NBLK = 126         # blocks per core
S1 = 32.0          # scale on W1/b1 (fp8 range use)
S2 = 512.0         # scale on W2/b2
NDUP = 62          # X rows corrected per parity

_compiled = {}
_last_in_maps = None


def _q8(a):
    return np.asarray(a, np.float32).astype(FP8)


def _build_program(nblk):
    f32 = mybir.dt.float32
    bf16 = mybir.dt.bfloat16
    fp8 = mybir.dt.float8e4
    Alu = mybir.AluOpType
    Act = mybir.ActivationFunctionType
    DR = mybir.MatmulPerfMode.DoubleRow
    mc_per_core = nblk * (BLK // MC)

    nc = bacc.Bacc("TRN2", target_bir_lowering=False, debug=False,
                   num_devices=N_CORES)

    xz_d = nc.dram_tensor("xz", [nblk, 128, 2 * BLK], fp8, kind="ExternalInput")
    # [parity][mhalf][p, ktile*128] fp8
    w1_d = nc.dram_tensor("w1", [2, 2, 128, 2 * 128], fp8, kind="ExternalInput")
    # [hi/lo][mhalf][p, ktile*128] fp8
    w2_d = nc.dram_tensor("w2", [2, 2, 128, 2 * 128], fp8, kind="ExternalInput")
    b2_d = nc.dram_tensor("b2", [2, 128, 1], f32, kind="ExternalInput")
    out_d = nc.dram_tensor("out", [128, 16 * nblk], f32, kind="ExternalOutput")

    with tile.TileContext(nc) as tc:
        with tc.tile_pool(name="consts", bufs=1) as cpool, \
             tc.tile_pool(name="work", bufs=4) as pool, \
             tc.tile_pool(name="psum", bufs=2, space="PSUM") as psum:

            w1t = {}
            for par in range(2):
                for m in range(2):
                    w = cpool.tile([128, 2 * 128], fp8, tag=f"w1_{par}{m}")
                    nc.sync.dma_start(w[:], w1_d[par, m])
                    w1t[par, m] = w[:].rearrange("p (k m) -> p k m", k=2)
            w2t = {}
            for t in range(2):
                for m in range(2):
                    w = cpool.tile([128, 2 * 128], fp8, tag=f"w2_{t}{m}")
                    nc.sync.dma_start(w[:], w2_d[t, m])
                    w2t[t, m] = w[:].rearrange("p (k m) -> p k m", k=2)
            b2a = cpool.tile([128, 1], f32, tag="b2a")
            b2b = cpool.tile([128, 1], f32, tag="b2b")
            nc.sync.dma_start(b2a[:], b2_d[0])
            nc.sync.dma_start(b2b[:], b2_d[1])

            out2 = cpool.tile([128, 16 * nblk], f32, tag="out2")

            def emit_dma(i):
                xz = pool.tile([128, 2 * BLK], fp8, tag="xz")
                nc.sync.dma_start(xz[:], xz_d[i])
                return xz

            def emit_l1(i, xz):
                ps1 = psum.tile([128, 2 * BLK], f32, tag="ps1")
                xzv = xz[:].rearrange("p (k c) -> p k c", k=2)
                par = i % 2
                nc.tensor.matmul(ps1[:, 0:BLK], w1t[par, 0], xzv,
                                 start=True, stop=True, perf_mode=DR)
                nc.tensor.matmul(ps1[:, BLK:2 * BLK], w1t[par, 1], xzv,
                                 start=True, stop=True, perf_mode=DR)
                return ps1

            def emit_relu1(i, ps1):
                # split across ACT/DVE for engine balance (bias rides in MM)
                h1 = pool.tile([128, 2 * BLK], fp8, tag="h1")
                nc.scalar.activation(h1[:, 0:BLK], ps1[:, 0:BLK], Act.Relu)
                nc.vector.tensor_scalar(h1[:, BLK:2 * BLK],
                                        ps1[:, BLK:2 * BLK], 0.0, None,
                                        op0=Alu.max)
                return h1

            def emit_l2(i, h1):
                ps2 = psum.tile([128, 2 * BLK], f32, tag="ps2")
                h1v = h1[:].rearrange("p (k c) -> p k c", k=2)
                lo = i % 2 == 0
                nc.tensor.matmul(ps2[:, 0:BLK], w2t[0, 0], h1v,
                                 start=True, stop=not lo, perf_mode=DR)
                if lo:
                    nc.tensor.matmul(ps2[:, 0:BLK], w2t[1, 0], h1v,
                                     start=False, stop=True, perf_mode=DR)
                nc.tensor.matmul(ps2[:, BLK:2 * BLK], w2t[0, 1], h1v,
                                 start=True, stop=not lo, perf_mode=DR)
                if lo:
                    nc.tensor.matmul(ps2[:, BLK:2 * BLK], w2t[1, 1], h1v,
                                     start=False, stop=True, perf_mode=DR)
                return ps2

            def emit_relu2(i, ps2):
                h2 = pool.tile([128, 2 * BLK], bf16, tag="h2")
                nc.scalar.activation(h2[:, 0:BLK], ps2[:, 0:BLK], Act.Relu,
                                     bias=b2a[:])
                nc.vector.tensor_scalar(h2[:, BLK:2 * BLK],
                                        ps2[:, BLK:2 * BLK], b2b[:], 0.0,
                                        op0=Alu.add, op1=Alu.max)
                return h2

            def emit_fold1(i, h2):
                # 16 groups of 64 -> 32 on GpSimd
                h2v = h2[:].rearrange("p (g t m) -> p g t m", t=2, m=MC // 2)
                h2f = pool.tile([128, BLK], bf16, tag="h2f")
                h2fv = h2f[:].rearrange("p (g m) -> p g m", m=MC // 2)
                nc.gpsimd.tensor_tensor(
                    h2fv, h2v[:, :, 0:1, :], h2v[:, :, 1:2, :], op=Alu.add)
                return h2f

            def emit_fold2(i, h2f):
                # 16 groups of 32 -> 16 on GpSimd
                h2fv = h2f[:].rearrange("p (g t m) -> p g t m", t=2, m=MC // 4)
                h2g = pool.tile([128, BLK // 2], bf16, tag="h2g")
                h2gv = h2g[:].rearrange("p (g m) -> p g m", m=MC // 4)
                nc.gpsimd.tensor_tensor(
                    h2gv, h2fv[:, :, 0:1, :], h2fv[:, :, 1:2, :], op=Alu.add)
                return h2g

            def emit_red(i, h2g):
                h2gv = h2g[:].rearrange("p (g m) -> p g m", m=MC // 4)
                nc.vector.tensor_reduce(
                    out2[:, i * 16:(i + 1) * 16], h2gv,
                    axis=mybir.AxisListType.X, op=Alu.add)

            # ---- software pipeline -------------------------------------
            xzs = {0: emit_dma(0), 1: emit_dma(1)}
            ps1s = {0: emit_l1(0, xzs.pop(0)), 1: emit_l1(1, xzs.pop(1))}
            ps2s = {}
            h2s = {}
            f1s = {}
            f2s = {}
            for i in range(nblk + 3):
                if i < nblk:
                    h1 = emit_relu1(i, ps1s.pop(i))      # ACT + DVE
                    if i + 2 < nblk:
                        xzs[i + 2] = emit_dma(i + 2)
                        ps1s[i + 2] = emit_l1(i + 2, xzs.pop(i + 2))  # PE
                    ps2s[i] = emit_l2(i, h1)             # PE
                if i - 1 in ps2s:
                    h2s[i - 1] = emit_relu2(i - 1, ps2s.pop(i - 1))  # ACT+DVE
                if i - 3 in f2s:
                    emit_red(i - 3, f2s.pop(i - 3))      # DVE
                if i - 2 in f1s:
                    f2s[i - 2] = emit_fold2(i - 2, f1s.pop(i - 2))   # GpSimd
                if i - 1 in h2s and i - 1 < nblk:
                    f1s[i - 1] = emit_fold1(i - 1, h2s.pop(i - 1))   # GpSimd
            for i in sorted(f1s):
                f2s[i] = emit_fold2(i, f1s.pop(i))
            for i in sorted(f2s):
                emit_red(i, f2s.pop(i))

            nc.sync.dma_start(out_d[:], out2[:])

    nc.compile()
    return nc


def _get_program(nblk):
    if nblk not in _compiled:
        _compiled[nblk] = _build_program(nblk)
    return _compiled[nblk]


def kernel(X, Z, W1, b1, W2, b2, W3, b3, cell_to_batch, sample_idx_batch):
    X = np.asarray(X)
    Z = np.asarray(Z)
    W1 = np.asarray(W1, dtype=np.float32)
    b1 = np.asarray(b1, dtype=np.float32)
    W2 = np.asarray(W2, dtype=np.float32)
    b2 = np.asarray(b2, dtype=np.float32)
    W3 = np.asarray(W3, dtype=np.float32)
    b3 = np.asarray(b3, dtype=np.float32)
    c2b = np.asarray(cell_to_batch).astype(np.int64)
    sib = np.asarray(sample_idx_batch).astype(np.int64)

    n = X.shape[0]
    nseg = sib.shape[0]
    seg = sib[c2b]

    # ---- minichunk assignment with per-segment parity balance -------------
    counts = np.bincount(seg, minlength=nseg).astype(np.int64)
    m_seg = (counts + MC - 1) // MC            # minichunks per segment
    M = int(m_seg.sum())
    nblk = NBLK
    while M > N_CORES * nblk * (BLK // MC):    # safety fallback, recompiles
        nblk += 2
    mc_per_core = nblk * (BLK // MC)
    n_mc = N_CORES * mc_per_core
    half_cap = n_mc // 2                        # even-block pool capacity

    # for each segment: alternate its minichunks between even/odd pools
    par_of = np.zeros(M, dtype=np.int64)        # parity of each (seg-ordered) mc
    mc_seg_label = np.zeros(M, dtype=np.int64)
    pos = 0
    tE = tO = 0
    for s in range(nseg):
        m = int(m_seg[s])
        if m == 0:
            continue
        start = 0 if tE <= tO else 1
        pars = (np.arange(m) + start) % 2
        par_of[pos:pos + m] = pars
        mc_seg_label[pos:pos + m] = s
        tE += int(np.sum(pars == 0))
        tO += int(np.sum(pars == 1))
        pos += m
    # index within parity pool, in segment order
    idx_in_pool = np.zeros(M, dtype=np.int64)
    isE = par_of == 0
    idx_in_pool[isE] = np.arange(int(isE.sum()))
    idx_in_pool[~isE] = np.arange(int((~isE).sum()))
    # pool index -> physical mc slot (core, block, mc_in_block)
    per_core_half = mc_per_core // 2            # mc slots of one parity per core
    core = idx_in_pool // per_core_half
    r = idx_in_pool % per_core_half
    block = 2 * (r // (BLK // MC)) + par_of
    mc_in_blk = r % (BLK // MC)
    mc_slot = (core * nblk + block) * (BLK // MC) + mc_in_blk
    assert mc_slot.max() < n_mc and idx_in_pool.max() < half_cap

    mc_label = np.full(n_mc, -1, dtype=np.int64)
    mc_real = np.zeros(n_mc, dtype=np.int64)
    mc_label[mc_slot] = mc_seg_label

    # cells -> slots
    order = np.argsort(seg, kind="stable")
    seg_sorted = seg[order]
    run_starts = np.concatenate([[0], np.cumsum(counts)])[:nseg]
    ranks = np.arange(n, dtype=np.int64) - run_starts[seg_sorted]
    # mc index within segment
    mc_of_cell = ranks // MC
    seg_mc_starts = np.concatenate([[0], np.cumsum(m_seg)])[:nseg]
    mc_id = seg_mc_starts[seg_sorted] + mc_of_cell      # index into M arrays
    slots = mc_slot[mc_id] * MC + (ranks % MC)
    np.add.at(mc_real, mc_slot[mc_id], 1)

    ntot = n_mc * MC

    # ---- data quantization ------------------------------------------------
    Xq = _q8(np.log1p(X, dtype=np.float32))             # [n, 128] fp8
    Zhi = _q8(Z)                                        # [B, 32]
    Zlo = _q8(16.0 * (Z - Zhi.astype(np.float32)))

    Xs = np.zeros((ntot, DX), dtype=FP8)
    Xs[slots] = Xq[order]
    Zs = np.zeros((ntot, 2 * DZ), dtype=FP8)
    Zs[slots, 0:DZ] = Zhi[seg_sorted]
    Zs[slots, DZ:2 * DZ] = Zlo[seg_sorted]

    # per-core/block transposed layout [core, blk, 128, 1024]
    xt = Xs.reshape(N_CORES, nblk, BLK, DX).transpose(0, 1, 3, 2)
    zt = Zs.reshape(N_CORES, nblk, BLK, 2 * DZ).transpose(0, 1, 3, 2)
    xz = np.empty((N_CORES, nblk, 128, 2 * BLK), dtype=FP8)
    xz[:, :, :, 0:BLK] = xt
    xz[:, :, 0:2 * DZ, BLK:2 * BLK] = zt
    # odd blocks: replace Z-data-lo rows with Zhi (for the W1z-lo correction)
    xz[:, 1::2, DZ:2 * DZ, BLK:2 * BLK] = zt[:, 1::2, 0:DZ, :]
    xz[:, :, BLK_ONES0, BLK:2 * BLK] = np.float32(1.0)
    xz[:, :, BLK_ONES1, BLK:2 * BLK] = np.float32(1.0)
    # X dup rows (parity-dependent)
    xz[:, 0::2, 2 * DZ + 2:128, BLK:2 * BLK] = xt[:, 0::2, 0:NDUP, :]
    xz[:, 1::2, 2 * DZ + 2:128, BLK:2 * BLK] = xt[:, 1::2, NDUP:2 * NDUP, :]

    # ---- weights ----------------------------------------------------------
    w1s = (S1 * W1).astype(np.float32)                  # [160, 256]
    w1x_hi = _q8(w1s[0:DX])                             # [128, 256]
    w1x_lo = _q8(2.0 * (w1s[0:DX] - w1x_hi.astype(np.float32)))
    w1z_hi = _q8(w1s[DX:DX + DZ])                       # [32, 256]
    # even blocks: Z-data-lo rows, scaled 2x (-> /8); odd: W1z quant error, 2x
    w1z_lo_even = (w1z_hi.astype(np.float32) / 8.0).astype(FP8)
    w1z_lo_odd = _q8(2.0 * (w1s[DX:DX + DZ] - w1z_hi.astype(np.float32)))
    b1s = (S1 * b1).astype(np.float32)
    b1hi = _q8(b1s)
    b1lo = _q8(b1s - b1hi.astype(np.float32))

    # stationary ktile rows [256, 256] then pack [parity][m][p, k*128+mc]
    w1q = np.zeros((2, 2, 128, 2 * 128), dtype=FP8)
    for par in range(2):
        st = np.zeros((256, H), dtype=FP8)
        st[0:128] = w1x_hi
        st[128 + 0:128 + DZ] = w1z_hi
        st[128 + DZ:128 + 2 * DZ] = w1z_lo_even if par == 0 else w1z_lo_odd
        st[128 + 2 * DZ] = b1hi
        st[128 + 2 * DZ + 1] = b1lo
        st[128 + 2 * DZ + 2:256] = w1x_lo[par * NDUP:(par + 1) * NDUP]
        for m in range(2):
            w1q[par, m] = (st.reshape(2, 128, H).transpose(1, 0, 2)
                           [:, :, m * 128:(m + 1) * 128].reshape(128, 256))

    w2s = (S2 * W2).astype(np.float32)
    t_hi = _q8(w2s)
    t_lo = _q8(2.0 * (w2s - t_hi.astype(np.float32)))
    w2q = np.zeros((2, 2, 128, 2 * 128), dtype=FP8)
    for t, term in enumerate((t_hi, t_lo)):
        for m in range(2):
            w2q[t, m] = (term.reshape(2, 128, H).transpose(1, 0, 2)
                         [:, :, m * 128:(m + 1) * 128].reshape(128, 256))
    b2d = np.ascontiguousarray((S1 * S2 * b2).reshape(2, 128, 1)) \
        .astype(np.float32)

    # ---- run on 8 cores ---------------------------------------------------
    nc = _get_program(nblk)
    in_maps = []
    for c in range(N_CORES):
        in_maps.append({"xz": xz[c], "w1": w1q, "w2": w2q, "b2": b2d})
    global _last_in_maps
    _last_in_maps = in_maps
    res = run_bass_kernel_spmd(nc, in_maps, list(range(N_CORES)))

    # ---- host epilogue ----------------------------------------------------
    per_core = []
    for c in range(N_CORES):
        o = res.results[c]["out"].reshape(128, nblk, 2, BLK // MC)
        per_core.append(np.concatenate(
            [o[:, :, 0, :].reshape(128, mc_per_core),
             o[:, :, 1, :].reshape(128, mc_per_core)], axis=0))
    sums = np.concatenate(per_core, axis=1)   # [256, n_mc], scaled S1*S2

    # analytic pad-cell contribution (X=0, Z=0, ones=1), parity-dependent
    pre1_pad = b1hi.astype(np.float32) + b1lo.astype(np.float32)  # [256]
    h1_pad = _q8(np.maximum(pre1_pad, 0.0)).astype(np.float32)
    w2hi_f = t_hi.astype(np.float32)
    w2lo_f = t_lo.astype(np.float32)
    b2s = (S1 * S2 * b2).astype(np.float32)
    v_pad_even = np.maximum(h1_pad @ (w2hi_f + w2lo_f) + b2s, 0.0) \
        .astype(BF16).astype(np.float32)
    v_pad_odd = np.maximum(h1_pad @ w2hi_f + b2s, 0.0) \
        .astype(BF16).astype(np.float32)
    blk_of_mc = (np.arange(n_mc) // (BLK // MC)) % nblk
    mc_parity = blk_of_mc % 2
    v_pad = np.where(mc_parity[None, :] == 0,
                     v_pad_even[:, None], v_pad_odd[:, None])
    sums = sums - v_pad * (MC - mc_real).astype(np.float32)[None, :]
    sums /= np.float32(S1 * S2)

    valid = mc_label >= 0
    S = np.zeros((nseg, H), dtype=np.float32)
    np.add.at(S, mc_label[valid], sums[:, valid].T)

    denom = np.maximum(counts, 1).astype(np.float32)[:, None]
    Y = S @ W3 / denom + b3[None, :]
    Y[counts == 0] = 0.0
    return Y.astype(np.float32)


BLK_ONES0 = 2 * DZ      # row 64 of ktile1 (ones -> b1 hi)
BLK_ONES1 = 2 * DZ + 1  # row 65 (ones -> b1 lo)


# revision 14
# speedup vs baseline: 1.0236x; 1.0236x over previous
"""Trainium2 Bass kernel for CompositionModel (gnn_message_passing).

Model: per-cell MLP over [log1p(X) ++ Z[cell_to_batch]] followed by a
segment-mean over batch labels.

v2 strategy (all-fp8 DoubleRow):
  * Host: log1p(X) computed on host, shipped as fp8. Cells sorted by segment,
    padded to 64-cell minichunks; minichunks assigned to even/odd blocks with
    per-segment parity balance (so alternate-block corrections average out
    per segment). Moving tile per 512-cell block is [128, 2, 512] fp8:
      ktile0       = Q8(log1p(X))                    (128 rows)
      ktile1[0:32] = Q8(Zc)            (hi)
      ktile1[32:64]= Q8(16*(Zc-hi))    (lo, exact-ish Z)
      ktile1[64]   = 1.0  -> stationary carries Q8(S1*b1)       (bias hi)
      ktile1[65]   = 1.0  -> stationary carries Q8(residual b1) (bias lo)
      ktile1[66:128] = dup of X rows [0:62] (even blk) / [62:124] (odd blk)
        -> stationary carries 2*(S1*W1x - Q8(S1*W1x)) : the W1 quantization
           error correction applied to half the rows on alternate blocks
           (first-order exact through the segment mean).
  * Device per block: L1 = 2 DR matmuls (K=256 incl. Z+bias+W1lo packed) ->
    ps1 [128,1024] f32 (2 banks) -> single DVE max(x,0) -> fp8 h1 ->
    L2 = 2 DR matmuls hi (+2 lo on even blocks) -> ps2 [128,1024] ->
    2 ACT relu+bias(b2 half) -> bf16 h2 -> GpSimd fold (64->32) ->
    DVE fold (32->16) -> DVE grouped tensor_reduce -> per-minichunk sums.
  * W3/b3 applied on host to the 512x256 segment sums; pad-cell contribution
    subtracted analytically (parity-dependent).
"""

import numpy as np
import ml_dtypes

import concourse.bacc as bacc
import concourse.mybir as mybir
import concourse.tile as tile
from concourse.bass_utils import run_bass_kernel_spmd

BF16 = ml_dtypes.bfloat16
FP8 = ml_dtypes.float8_e4m3fn

N_CORES = 8
DX = 128
DZ = 32
H = 256
B = 512
MC = 64            # minichunk: cells per single-segment group
BLK = 512          # cells per device block
NBLK = 126         # blocks per core
S1 = 32.0          # scale on W1/b1 (fp8 range use)
S2 = 512.0         # scale on W2/b2
NDUP = 62          # X rows corrected per parity

_compiled = {}
_last_in_maps = None


def _q8(a):
    return np.asarray(a, np.float32).astype(FP8)


def _build_program(nblk):
    f32 = mybir.dt.float32
    bf16 = mybir.dt.bfloat16
    fp8 = mybir.dt.float8e4
    Alu = mybir.AluOpType
    Act = mybir.ActivationFunctionType
    DR = mybir.MatmulPerfMode.DoubleRow
    mc_per_core = nblk * (BLK // MC)

    nc = bacc.Bacc("TRN2", target_bir_lowering=False, debug=False,
                   num_devices=N_CORES)

    xz_d = nc.dram_tensor("xz", [nblk, 128, 2 * BLK], fp8, kind="ExternalInput")
    # w1[parity][mhalf] ++ w2[hi/lo][mhalf], each [p, ktile*128], one DMA
    wq_d = nc.dram_tensor("wq", [128, 8 * 2 * 128], fp8, kind="ExternalInput")
    b2_d = nc.dram_tensor("b2", [2, 128, 1], f32, kind="ExternalInput")
    out_d = nc.dram_tensor("out", [128, 16 * nblk], f32, kind="ExternalOutput")

    with tile.TileContext(nc) as tc:
        with tc.tile_pool(name="consts", bufs=1) as cpool, \
             tc.tile_pool(name="work", bufs=4) as pool, \
             tc.tile_pool(name="psum", bufs=2, space="PSUM") as psum:

            wq = cpool.tile([128, 8 * 2 * 128], fp8, tag="wq")
            nc.sync.dma_start(wq[:], wq_d[:])
            wv = wq[:].rearrange("p (i k m) -> p i k m", i=8, k=2)
            w1t = {(par, m): wv[:, 2 * par + m]
                   for par in range(2) for m in range(2)}
            w2t = {(t, m): wv[:, 4 + 2 * t + m]
                   for t in range(2) for m in range(2)}
            b2a = cpool.tile([128, 1], f32, tag="b2a")
            b2b = cpool.tile([128, 1], f32, tag="b2b")
            nc.sync.dma_start(b2a[:], b2_d[0])
            nc.sync.dma_start(b2b[:], b2_d[1])

            out2 = cpool.tile([128, 16 * nblk], f32, tag="out2")

            def emit_dma(i):
                xz = pool.tile([128, 2 * BLK], fp8, tag="xz", bufs=6)
                nc.sync.dma_start(xz[:], xz_d[i])
                return xz

            def emit_l1(i, xz):
                ps1 = psum.tile([128, 2 * BLK], f32, tag="ps1")
                xzv = xz[:].rearrange("p (k c) -> p k c", k=2)
                par = i % 2
                nc.tensor.matmul(ps1[:, 0:BLK], w1t[par, 0], xzv,
                                 start=True, stop=True, perf_mode=DR)
                nc.tensor.matmul(ps1[:, BLK:2 * BLK], w1t[par, 1], xzv,
                                 start=True, stop=True, perf_mode=DR)
                return ps1

            def emit_relu1(i, ps1):
                # split across ACT/DVE for engine balance (bias rides in MM)
                h1 = pool.tile([128, 2 * BLK], fp8, tag="h1")
                cut = 640
                nc.scalar.activation(h1[:, 0:cut], ps1[:, 0:cut], Act.Relu)
                nc.vector.tensor_scalar(h1[:, cut:2 * BLK],
                                        ps1[:, cut:2 * BLK], 0.0, None,
                                        op0=Alu.max)
                return h1

            def emit_l2(i, h1):
                ps2 = psum.tile([128, 2 * BLK], f32, tag="ps2")
                h1v = h1[:].rearrange("p (k c) -> p k c", k=2)
                lo = i % 2 == 0
                nc.tensor.matmul(ps2[:, 0:BLK], w2t[0, 0], h1v,
                                 start=True, stop=not lo, perf_mode=DR)
                if lo:
                    nc.tensor.matmul(ps2[:, 0:BLK], w2t[1, 0], h1v,
                                     start=False, stop=True, perf_mode=DR)
                nc.tensor.matmul(ps2[:, BLK:2 * BLK], w2t[0, 1], h1v,
                                 start=True, stop=not lo, perf_mode=DR)
                if lo:
                    nc.tensor.matmul(ps2[:, BLK:2 * BLK], w2t[1, 1], h1v,
                                     start=False, stop=True, perf_mode=DR)
                return ps2

            def emit_relu2(i, ps2):
                h2 = pool.tile([128, 2 * BLK], bf16, tag="h2")
                nc.scalar.activation(h2[:, 0:BLK], ps2[:, 0:BLK], Act.Relu,
                                     bias=b2a[:])
                nc.vector.tensor_scalar(h2[:, BLK:2 * BLK],
                                        ps2[:, BLK:2 * BLK], b2b[:], 0.0,
                                        op0=Alu.add, op1=Alu.max)
                return h2

            def emit_fold1(i, h2):
                # 16 groups of 64 -> 32 on GpSimd
                h2v = h2[:].rearrange("p (g t m) -> p g t m", t=2, m=MC // 2)
                h2f = pool.tile([128, BLK], bf16, tag="h2f")
                h2fv = h2f[:].rearrange("p (g m) -> p g m", m=MC // 2)
                nc.gpsimd.tensor_tensor(
                    h2fv, h2v[:, :, 0:1, :], h2v[:, :, 1:2, :], op=Alu.add)
                return h2f

            def emit_fold2(i, h2f):
                # 16 groups of 32 -> 16 on GpSimd
                h2fv = h2f[:].rearrange("p (g t m) -> p g t m", t=2, m=MC // 4)
                h2g = pool.tile([128, BLK // 2], bf16, tag="h2g")
                h2gv = h2g[:].rearrange("p (g m) -> p g m", m=MC // 4)
                nc.gpsimd.tensor_tensor(
                    h2gv, h2fv[:, :, 0:1, :], h2fv[:, :, 1:2, :], op=Alu.add)
                return h2g

            def emit_red(i, h2g):
                h2gv = h2g[:].rearrange("p (g m) -> p g m", m=MC // 4)
                nc.vector.tensor_reduce(
                    out2[:, i * 16:(i + 1) * 16], h2gv,
                    axis=mybir.AxisListType.X, op=Alu.add)

            # ---- software pipeline -------------------------------------
            xzs = {j: emit_dma(j) for j in range(min(4, nblk))}
            ps1s = {0: emit_l1(0, xzs.pop(0)), 1: emit_l1(1, xzs.pop(1))}
            ps2s = {}
            h2s = {}
            f1s = {}
            f2s = {}
            for i in range(nblk + 3):
                if i < nblk:
                    h1 = emit_relu1(i, ps1s.pop(i))      # ACT + DVE
                    ps2s[i] = emit_l2(i, h1)             # PE
                    if i + 4 < nblk:
                        xzs[i + 4] = emit_dma(i + 4)
                    if i + 2 < nblk:
                        ps1s[i + 2] = emit_l1(i + 2, xzs.pop(i + 2))  # PE
                if i - 1 in ps2s:
                    h2s[i - 1] = emit_relu2(i - 1, ps2s.pop(i - 1))  # ACT+DVE
                if i - 3 in f2s:
                    emit_red(i - 3, f2s.pop(i - 3))      # DVE
                if i - 2 in f1s:
                    f2s[i - 2] = emit_fold2(i - 2, f1s.pop(i - 2))   # GpSimd
                if i - 1 in h2s and i - 1 < nblk:
                    f1s[i - 1] = emit_fold1(i - 1, h2s.pop(i - 1))   # GpSimd
            for i in sorted(f1s):
                f2s[i] = emit_fold2(i, f1s.pop(i))
            for i in sorted(f2s):
                emit_red(i, f2s.pop(i))

            nc.sync.dma_start(out_d[:], out2[:])

    nc.compile()
    return nc


def _get_program(nblk):
    if nblk not in _compiled:
        _compiled[nblk] = _build_program(nblk)
    return _compiled[nblk]


def kernel(X, Z, W1, b1, W2, b2, W3, b3, cell_to_batch, sample_idx_batch):
    X = np.asarray(X)
    Z = np.asarray(Z)
    W1 = np.asarray(W1, dtype=np.float32)
    b1 = np.asarray(b1, dtype=np.float32)
    W2 = np.asarray(W2, dtype=np.float32)
    b2 = np.asarray(b2, dtype=np.float32)
    W3 = np.asarray(W3, dtype=np.float32)
    b3 = np.asarray(b3, dtype=np.float32)
    c2b = np.asarray(cell_to_batch).astype(np.int64)
    sib = np.asarray(sample_idx_batch).astype(np.int64)

    n = X.shape[0]
    nseg = sib.shape[0]
    seg = sib[c2b]

    # ---- minichunk assignment with per-segment parity balance -------------
    counts = np.bincount(seg, minlength=nseg).astype(np.int64)
    m_seg = (counts + MC - 1) // MC            # minichunks per segment
    M = int(m_seg.sum())
    nblk = NBLK
    while M > N_CORES * nblk * (BLK // MC):    # safety fallback, recompiles
        nblk += 2
    mc_per_core = nblk * (BLK // MC)
    n_mc = N_CORES * mc_per_core
    half_cap = n_mc // 2                        # even-block pool capacity

    # for each segment: alternate its minichunks between even/odd pools
    par_of = np.zeros(M, dtype=np.int64)        # parity of each (seg-ordered) mc
    mc_seg_label = np.zeros(M, dtype=np.int64)
    pos = 0
    tE = tO = 0
    for s in range(nseg):
        m = int(m_seg[s])
        if m == 0:
            continue
        start = 0 if tE <= tO else 1
        pars = (np.arange(m) + start) % 2
        par_of[pos:pos + m] = pars
        mc_seg_label[pos:pos + m] = s
        tE += int(np.sum(pars == 0))
        tO += int(np.sum(pars == 1))
        pos += m
    # index within parity pool, in segment order
    idx_in_pool = np.zeros(M, dtype=np.int64)
    isE = par_of == 0
    idx_in_pool[isE] = np.arange(int(isE.sum()))
    idx_in_pool[~isE] = np.arange(int((~isE).sum()))
    # pool index -> physical mc slot (core, block, mc_in_block)
    per_core_half = mc_per_core // 2            # mc slots of one parity per core
    core = idx_in_pool // per_core_half
    r = idx_in_pool % per_core_half
    block = 2 * (r // (BLK // MC)) + par_of
    mc_in_blk = r % (BLK // MC)
    mc_slot = (core * nblk + block) * (BLK // MC) + mc_in_blk
    assert mc_slot.max() < n_mc and idx_in_pool.max() < half_cap

    mc_label = np.full(n_mc, -1, dtype=np.int64)
    mc_real = np.zeros(n_mc, dtype=np.int64)
    mc_label[mc_slot] = mc_seg_label

    # cells -> slots
    order = np.argsort(seg, kind="stable")
    seg_sorted = seg[order]
    run_starts = np.concatenate([[0], np.cumsum(counts)])[:nseg]
    ranks = np.arange(n, dtype=np.int64) - run_starts[seg_sorted]
    # mc index within segment
    mc_of_cell = ranks // MC
    seg_mc_starts = np.concatenate([[0], np.cumsum(m_seg)])[:nseg]
    mc_id = seg_mc_starts[seg_sorted] + mc_of_cell      # index into M arrays
    slots = mc_slot[mc_id] * MC + (ranks % MC)
    np.add.at(mc_real, mc_slot[mc_id], 1)

    ntot = n_mc * MC

    # ---- data quantization ------------------------------------------------
    Xq = _q8(np.log1p(X, dtype=np.float32))             # [n, 128] fp8
    Zhi = _q8(Z)                                        # [B, 32]
    Zlo = _q8(16.0 * (Z - Zhi.astype(np.float32)))

    Xs = np.zeros((ntot, DX), dtype=FP8)
    Xs[slots] = Xq[order]
    Zs = np.zeros((ntot, 2 * DZ), dtype=FP8)
    Zs[slots, 0:DZ] = Zhi[seg_sorted]
    Zs[slots, DZ:2 * DZ] = Zlo[seg_sorted]

    # per-core/block transposed layout [core, blk, 128, 1024]
    xt = Xs.reshape(N_CORES, nblk, BLK, DX).transpose(0, 1, 3, 2)
    zt = Zs.reshape(N_CORES, nblk, BLK, 2 * DZ).transpose(0, 1, 3, 2)
    xz = np.empty((N_CORES, nblk, 128, 2 * BLK), dtype=FP8)
    xz[:, :, :, 0:BLK] = xt
    xz[:, :, 0:2 * DZ, BLK:2 * BLK] = zt
    # odd blocks: replace Z-data-lo rows with Zhi (for the W1z-lo correction)
    xz[:, 1::2, DZ:2 * DZ, BLK:2 * BLK] = zt[:, 1::2, 0:DZ, :]
    xz[:, :, BLK_ONES0, BLK:2 * BLK] = np.float32(1.0)
    xz[:, :, BLK_ONES1, BLK:2 * BLK] = np.float32(1.0)
    # X dup rows (parity-dependent)
    xz[:, 0::2, 2 * DZ + 2:128, BLK:2 * BLK] = xt[:, 0::2, 0:NDUP, :]
    xz[:, 1::2, 2 * DZ + 2:128, BLK:2 * BLK] = xt[:, 1::2, NDUP:2 * NDUP, :]

    # ---- weights ----------------------------------------------------------
    w1s = (S1 * W1).astype(np.float32)                  # [160, 256]
    w1x_hi = _q8(w1s[0:DX])                             # [128, 256]
    w1x_lo = _q8(2.0 * (w1s[0:DX] - w1x_hi.astype(np.float32)))
    w1z_hi = _q8(w1s[DX:DX + DZ])                       # [32, 256]
    # even blocks: Z-data-lo rows, scaled 2x (-> /8); odd: W1z quant error, 2x
    w1z_lo_even = (w1z_hi.astype(np.float32) / 8.0).astype(FP8)
    w1z_lo_odd = _q8(2.0 * (w1s[DX:DX + DZ] - w1z_hi.astype(np.float32)))
    b1s = (S1 * b1).astype(np.float32)
    b1hi = _q8(b1s)
    b1lo = _q8(b1s - b1hi.astype(np.float32))

    # stationary ktile rows [256, 256] then pack [parity][m][p, k*128+mc]
    w1q = np.zeros((2, 2, 128, 2 * 128), dtype=FP8)
    for par in range(2):
        st = np.zeros((256, H), dtype=FP8)
        st[0:128] = w1x_hi
        st[128 + 0:128 + DZ] = w1z_hi
        st[128 + DZ:128 + 2 * DZ] = w1z_lo_even if par == 0 else w1z_lo_odd
        st[128 + 2 * DZ] = b1hi
        st[128 + 2 * DZ + 1] = b1lo
        st[128 + 2 * DZ + 2:256] = w1x_lo[par * NDUP:(par + 1) * NDUP]
        for m in range(2):
            w1q[par, m] = (st.reshape(2, 128, H).transpose(1, 0, 2)
                           [:, :, m * 128:(m + 1) * 128].reshape(128, 256))

    w2s = (S2 * W2).astype(np.float32)
    t_hi = _q8(w2s)
    t_lo = _q8(2.0 * (w2s - t_hi.astype(np.float32)))
    w2q = np.zeros((2, 2, 128, 2 * 128), dtype=FP8)
    for t, term in enumerate((t_hi, t_lo)):
        for m in range(2):
            w2q[t, m] = (term.reshape(2, 128, H).transpose(1, 0, 2)
                         [:, :, m * 128:(m + 1) * 128].reshape(128, 256))
    b2d = np.ascontiguousarray((S1 * S2 * b2).reshape(2, 128, 1)) \
        .astype(np.float32)

    wq_host = np.concatenate(
        [w1q[0, 0], w1q[0, 1], w1q[1, 0], w1q[1, 1],
         w2q[0, 0], w2q[0, 1], w2q[1, 0], w2q[1, 1]], axis=1)

    # ---- run on 8 cores ---------------------------------------------------
    nc = _get_program(nblk)
    in_maps = []
    for c in range(N_CORES):
        in_maps.append({"xz": xz[c], "wq": wq_host, "b2": b2d})
    global _last_in_maps
    _last_in_maps = in_maps
    res = run_bass_kernel_spmd(nc, in_maps, list(range(N_CORES)))

    # ---- host epilogue ----------------------------------------------------
    per_core = []
    for c in range(N_CORES):
        o = res.results[c]["out"].reshape(128, nblk, 2, BLK // MC)
        per_core.append(np.concatenate(
            [o[:, :, 0, :].reshape(128, mc_per_core),
             o[:, :, 1, :].reshape(128, mc_per_core)], axis=0))
    sums = np.concatenate(per_core, axis=1)   # [256, n_mc], scaled S1*S2

    # analytic pad-cell contribution (X=0, Z=0, ones=1), parity-dependent
    pre1_pad = b1hi.astype(np.float32) + b1lo.astype(np.float32)  # [256]
    h1_pad = _q8(np.maximum(pre1_pad, 0.0)).astype(np.float32)
    w2hi_f = t_hi.astype(np.float32)
    w2lo_f = t_lo.astype(np.float32)
    b2s = (S1 * S2 * b2).astype(np.float32)
    v_pad_even = np.maximum(h1_pad @ (w2hi_f + w2lo_f) + b2s, 0.0) \
        .astype(BF16).astype(np.float32)
    v_pad_odd = np.maximum(h1_pad @ w2hi_f + b2s, 0.0) \
        .astype(BF16).astype(np.float32)
    blk_of_mc = (np.arange(n_mc) // (BLK // MC)) % nblk
    mc_parity = blk_of_mc % 2
    v_pad = np.where(mc_parity[None, :] == 0,
                     v_pad_even[:, None], v_pad_odd[:, None])
    sums = sums - v_pad * (MC - mc_real).astype(np.float32)[None, :]
    sums /= np.float32(S1 * S2)

    valid = mc_label >= 0
    S = np.zeros((nseg, H), dtype=np.float32)
    np.add.at(S, mc_label[valid], sums[:, valid].T)

    denom = np.maximum(counts, 1).astype(np.float32)[:, None]
    Y = S @ W3 / denom + b3[None, :]
    Y[counts == 0] = 0.0
    return Y.astype(np.float32)


BLK_ONES0 = 2 * DZ      # row 64 of ktile1 (ones -> b1 hi)
BLK_ONES1 = 2 * DZ + 1  # row 65 (ones -> b1 lo)


# revision 17
# speedup vs baseline: 1.0286x; 1.0050x over previous
"""Trainium2 Bass kernel for CompositionModel (gnn_message_passing).

Model: per-cell MLP over [log1p(X) ++ Z[cell_to_batch]] followed by a
segment-mean over batch labels.

v2 strategy (all-fp8 DoubleRow):
  * Host: log1p(X) computed on host, shipped as fp8. Cells sorted by segment,
    padded to 64-cell minichunks; minichunks assigned to even/odd blocks with
    per-segment parity balance (so alternate-block corrections average out
    per segment). Moving tile per 512-cell block is [128, 2, 512] fp8:
      ktile0       = Q8(log1p(X))                    (128 rows)
      ktile1[0:32] = Q8(Zc)            (hi)
      ktile1[32:64]= Q8(16*(Zc-hi))    (lo, exact-ish Z)
      ktile1[64]   = 1.0  -> stationary carries Q8(S1*b1)       (bias hi)
      ktile1[65]   = 1.0  -> stationary carries Q8(residual b1) (bias lo)
      ktile1[66:128] = dup of X rows [0:62] (even blk) / [62:124] (odd blk)
        -> stationary carries 2*(S1*W1x - Q8(S1*W1x)) : the W1 quantization
           error correction applied to half the rows on alternate blocks
           (first-order exact through the segment mean).
  * Device per block: L1 = 2 DR matmuls (K=256 incl. Z+bias+W1lo packed) ->
    ps1 [128,1024] f32 (2 banks) -> single DVE max(x,0) -> fp8 h1 ->
    L2 = 2 DR matmuls hi (+2 lo on even blocks) -> ps2 [128,1024] ->
    2 ACT relu+bias(b2 half) -> bf16 h2 -> GpSimd fold (64->32) ->
    DVE fold (32->16) -> DVE grouped tensor_reduce -> per-minichunk sums.
  * W3/b3 applied on host to the 512x256 segment sums; pad-cell contribution
    subtracted analytically (parity-dependent).
"""

import numpy as np
import ml_dtypes

import concourse.bacc as bacc
import concourse.mybir as mybir
import concourse.tile as tile
from concourse.bass_utils import run_bass_kernel_spmd

BF16 = ml_dtypes.bfloat16
FP8 = ml_dtypes.float8_e4m3fn

N_CORES = 8
DX = 128
DZ = 32
H = 256
B = 512
MC = 64            # minichunk: cells per single-segment group
BLK = 512          # cells per device block
NBLK = 126         # blocks per core
S1 = 32.0          # scale on W1/b1 (fp8 range use)
S2 = 512.0         # scale on W2/b2
NDUP = 62          # X rows corrected per parity

_compiled = {}
_last_in_maps = None


def _q8(a):
    return np.asarray(a, np.float32).astype(FP8)


def _build_program(nblk):
    f32 = mybir.dt.float32
    bf16 = mybir.dt.bfloat16
    fp8 = mybir.dt.float8e4
    Alu = mybir.AluOpType
    Act = mybir.ActivationFunctionType
    DR = mybir.MatmulPerfMode.DoubleRow
    mc_per_core = nblk * (BLK // MC)

    nc = bacc.Bacc("TRN2", target_bir_lowering=False, debug=False,
                   num_devices=N_CORES)

    xz_d = nc.dram_tensor("xz", [nblk, 128, 2 * BLK], fp8, kind="ExternalInput")
    # w1[parity][mhalf] ++ w2[hi/lo][mhalf], each [p, ktile*128], one DMA
    wq_d = nc.dram_tensor("wq", [128, 8 * 2 * 128], fp8, kind="ExternalInput")
    b2_d = nc.dram_tensor("b2", [2, 128, 1], f32, kind="ExternalInput")
    out_d = nc.dram_tensor("out", [128, 16 * nblk], f32, kind="ExternalOutput")

    with tile.TileContext(nc) as tc:
        with tc.tile_pool(name="consts", bufs=1) as cpool, \
             tc.tile_pool(name="work", bufs=4) as pool, \
             tc.tile_pool(name="psum", bufs=2, space="PSUM") as psum:

            wq = cpool.tile([128, 8 * 2 * 128], fp8, tag="wq")
            nc.sync.dma_start(wq[:], wq_d[:])
            wv = wq[:].rearrange("p (i k m) -> p i k m", i=8, k=2)
            w1t = {(par, m): wv[:, 2 * par + m]
                   for par in range(2) for m in range(2)}
            w2t = {(t, m): wv[:, 4 + 2 * t + m]
                   for t in range(2) for m in range(2)}
            b2a = cpool.tile([128, 1], f32, tag="b2a")
            b2b = cpool.tile([128, 1], f32, tag="b2b")
            nc.sync.dma_start(b2a[:], b2_d[0])
            nc.sync.dma_start(b2b[:], b2_d[1])

            out2 = cpool.tile([128, 16 * nblk], f32, tag="out2")

            def emit_dma(i):
                xz = pool.tile([128, 2 * BLK], fp8, tag="xz", bufs=6)
                nc.sync.dma_start(xz[:], xz_d[i])
                return xz

            def emit_l1(i, xz):
                ps1 = psum.tile([128, 2 * BLK], f32, tag="ps1")
                xzv = xz[:].rearrange("p (k c) -> p k c", k=2)
                par = i % 2
                nc.tensor.matmul(ps1[:, 0:BLK], w1t[par, 0], xzv,
                                 start=True, stop=True, perf_mode=DR)
                nc.tensor.matmul(ps1[:, BLK:2 * BLK], w1t[par, 1], xzv,
                                 start=True, stop=True, perf_mode=DR)
                return ps1

            def emit_relu1(i, ps1):
                # split across ACT/DVE for engine balance (bias rides in MM)
                h1 = pool.tile([128, 2 * BLK], fp8, tag="h1")
                cut = 512
                nc.scalar.activation(h1[:, 0:cut], ps1[:, 0:cut], Act.Relu)
                nc.vector.tensor_scalar(h1[:, cut:2 * BLK],
                                        ps1[:, cut:2 * BLK], 0.0, None,
                                        op0=Alu.max)
                return h1

            def emit_l2(i, h1):
                ps2 = psum.tile([128, 2 * BLK], f32, tag="ps2")
                h1v = h1[:].rearrange("p (k c) -> p k c", k=2)
                lo = i % 2 == 0
                nc.tensor.matmul(ps2[:, 0:BLK], w2t[0, 0], h1v,
                                 start=True, stop=not lo, perf_mode=DR)
                if lo:
                    nc.tensor.matmul(ps2[:, 0:BLK], w2t[1, 0], h1v,
                                     start=False, stop=True, perf_mode=DR)
                nc.tensor.matmul(ps2[:, BLK:2 * BLK], w2t[0, 1], h1v,
                                 start=True, stop=not lo, perf_mode=DR)
                if lo:
                    nc.tensor.matmul(ps2[:, BLK:2 * BLK], w2t[1, 1], h1v,
                                     start=False, stop=True, perf_mode=DR)
                return ps2

            def emit_relu2(i, ps2):
                h2 = pool.tile([128, 2 * BLK], bf16, tag="h2")
                nc.scalar.activation(h2[:, 0:BLK], ps2[:, 0:BLK], Act.Relu,
                                     bias=b2a[:])
                nc.vector.tensor_scalar(h2[:, BLK:2 * BLK],
                                        ps2[:, BLK:2 * BLK], b2b[:], 0.0,
                                        op0=Alu.add, op1=Alu.max)
                return h2

            def emit_fold1(i, h2):
                # 16 groups of 64 -> 32 on GpSimd
                h2v = h2[:].rearrange("p (g t m) -> p g t m", t=2, m=MC // 2)
                h2f = pool.tile([128, BLK], bf16, tag="h2f")
                h2fv = h2f[:].rearrange("p (g m) -> p g m", m=MC // 2)
                nc.gpsimd.tensor_tensor(
                    h2fv, h2v[:, :, 0:1, :], h2v[:, :, 1:2, :], op=Alu.add)
                return h2f

            def emit_fold2(i, h2f):
                # 16 groups of 32 -> 16 on GpSimd
                h2fv = h2f[:].rearrange("p (g t m) -> p g t m", t=2, m=MC // 4)
                h2g = pool.tile([128, BLK // 2], bf16, tag="h2g")
                h2gv = h2g[:].rearrange("p (g m) -> p g m", m=MC // 4)
                nc.gpsimd.tensor_tensor(
                    h2gv, h2fv[:, :, 0:1, :], h2fv[:, :, 1:2, :], op=Alu.add)
                return h2g

            def emit_red(i, h2g):
                h2gv = h2g[:].rearrange("p (g m) -> p g m", m=MC // 4)
                nc.vector.tensor_reduce(
                    out2[:, i * 16:(i + 1) * 16], h2gv,
                    axis=mybir.AxisListType.X, op=Alu.add)
                # stream finished output chunks out early to shorten the tail
                if (i + 1) % 16 == 0:
                    nc.sync.dma_start(out_d[:, (i - 15) * 16:(i + 1) * 16],
                                      out2[:, (i - 15) * 16:(i + 1) * 16])
                elif i == nblk - 1:
                    base = (nblk // 16) * 16
                    nc.sync.dma_start(out_d[:, base * 16:nblk * 16],
                                      out2[:, base * 16:nblk * 16])

            # ---- software pipeline -------------------------------------
            xzs = {j: emit_dma(j) for j in range(min(4, nblk))}
            ps1s = {0: emit_l1(0, xzs.pop(0)), 1: emit_l1(1, xzs.pop(1))}
            ps2s = {}
            h2s = {}
            f1s = {}
            f2s = {}
            for i in range(nblk + 3):
                if i < nblk:
                    h1 = emit_relu1(i, ps1s.pop(i))      # ACT + DVE
                    ps2s[i] = emit_l2(i, h1)             # PE
                    if i + 4 < nblk:
                        xzs[i + 4] = emit_dma(i + 4)
                    if i + 2 < nblk:
                        ps1s[i + 2] = emit_l1(i + 2, xzs.pop(i + 2))  # PE
                if i - 1 in ps2s:
                    h2s[i - 1] = emit_relu2(i - 1, ps2s.pop(i - 1))  # ACT+DVE
                if i - 3 in f2s:
                    emit_red(i - 3, f2s.pop(i - 3))      # DVE
                if i - 2 in f1s:
                    f2s[i - 2] = emit_fold2(i - 2, f1s.pop(i - 2))   # GpSimd
                if i - 1 in h2s and i - 1 < nblk:
                    f1s[i - 1] = emit_fold1(i - 1, h2s.pop(i - 1))   # GpSimd
            for i in sorted(f1s):
                f2s[i] = emit_fold2(i, f1s.pop(i))
            for i in sorted(f2s):
                emit_red(i, f2s.pop(i))

    nc.compile()
    return nc


def _get_program(nblk):
    if nblk not in _compiled:
        _compiled[nblk] = _build_program(nblk)
    return _compiled[nblk]


def kernel(X, Z, W1, b1, W2, b2, W3, b3, cell_to_batch, sample_idx_batch):
    X = np.asarray(X)
    Z = np.asarray(Z)
    W1 = np.asarray(W1, dtype=np.float32)
    b1 = np.asarray(b1, dtype=np.float32)
    W2 = np.asarray(W2, dtype=np.float32)
    b2 = np.asarray(b2, dtype=np.float32)
    W3 = np.asarray(W3, dtype=np.float32)
    b3 = np.asarray(b3, dtype=np.float32)
    c2b = np.asarray(cell_to_batch).astype(np.int64)
    sib = np.asarray(sample_idx_batch).astype(np.int64)

    n = X.shape[0]
    nseg = sib.shape[0]
    seg = sib[c2b]

    # ---- minichunk assignment with per-segment parity balance -------------
    counts = np.bincount(seg, minlength=nseg).astype(np.int64)
    m_seg = (counts + MC - 1) // MC            # minichunks per segment
    M = int(m_seg.sum())
    nblk = NBLK
    while M > N_CORES * nblk * (BLK // MC):    # safety fallback, recompiles
        nblk += 2
    mc_per_core = nblk * (BLK // MC)
    n_mc = N_CORES * mc_per_core
    half_cap = n_mc // 2                        # even-block pool capacity

    # for each segment: alternate its minichunks between even/odd pools
    par_of = np.zeros(M, dtype=np.int64)        # parity of each (seg-ordered) mc
    mc_seg_label = np.zeros(M, dtype=np.int64)
    pos = 0
    tE = tO = 0
    for s in range(nseg):
        m = int(m_seg[s])
        if m == 0:
            continue
        start = 0 if tE <= tO else 1
        pars = (np.arange(m) + start) % 2
        par_of[pos:pos + m] = pars
        mc_seg_label[pos:pos + m] = s
        tE += int(np.sum(pars == 0))
        tO += int(np.sum(pars == 1))
        pos += m
    # index within parity pool, in segment order
    idx_in_pool = np.zeros(M, dtype=np.int64)
    isE = par_of == 0
    idx_in_pool[isE] = np.arange(int(isE.sum()))
    idx_in_pool[~isE] = np.arange(int((~isE).sum()))
    # pool index -> physical mc slot (core, block, mc_in_block)
    per_core_half = mc_per_core // 2            # mc slots of one parity per core
    core = idx_in_pool // per_core_half
    r = idx_in_pool % per_core_half
    block = 2 * (r // (BLK // MC)) + par_of
    mc_in_blk = r % (BLK // MC)
    mc_slot = (core * nblk + block) * (BLK // MC) + mc_in_blk
    assert mc_slot.max() < n_mc and idx_in_pool.max() < half_cap

    mc_label = np.full(n_mc, -1, dtype=np.int64)
    mc_real = np.zeros(n_mc, dtype=np.int64)
    mc_label[mc_slot] = mc_seg_label

    # cells -> slots
    order = np.argsort(seg, kind="stable")
    seg_sorted = seg[order]
    run_starts = np.concatenate([[0], np.cumsum(counts)])[:nseg]
    ranks = np.arange(n, dtype=np.int64) - run_starts[seg_sorted]
    # mc index within segment
    mc_of_cell = ranks // MC
    seg_mc_starts = np.concatenate([[0], np.cumsum(m_seg)])[:nseg]
    mc_id = seg_mc_starts[seg_sorted] + mc_of_cell      # index into M arrays
    slots = mc_slot[mc_id] * MC + (ranks % MC)
    np.add.at(mc_real, mc_slot[mc_id], 1)

    ntot = n_mc * MC

    # ---- data quantization ------------------------------------------------
    Xq = _q8(np.log1p(X, dtype=np.float32))             # [n, 128] fp8
    Zhi = _q8(Z)                                        # [B, 32]
    Zlo = _q8(16.0 * (Z - Zhi.astype(np.float32)))

    Xs = np.zeros((ntot, DX), dtype=FP8)
    Xs[slots] = Xq[order]
    Zs = np.zeros((ntot, 2 * DZ), dtype=FP8)
    Zs[slots, 0:DZ] = Zhi[seg_sorted]
    Zs[slots, DZ:2 * DZ] = Zlo[seg_sorted]

    # per-core/block transposed layout [core, blk, 128, 1024]
    xt = Xs.reshape(N_CORES, nblk, BLK, DX).transpose(0, 1, 3, 2)
    zt = Zs.reshape(N_CORES, nblk, BLK, 2 * DZ).transpose(0, 1, 3, 2)
    xz = np.empty((N_CORES, nblk, 128, 2 * BLK), dtype=FP8)
    xz[:, :, :, 0:BLK] = xt
    xz[:, :, 0:2 * DZ, BLK:2 * BLK] = zt
    # odd blocks: replace Z-data-lo rows with Zhi (for the W1z-lo correction)
    xz[:, 1::2, DZ:2 * DZ, BLK:2 * BLK] = zt[:, 1::2, 0:DZ, :]
    xz[:, :, BLK_ONES0, BLK:2 * BLK] = np.float32(1.0)
    xz[:, :, BLK_ONES1, BLK:2 * BLK] = np.float32(1.0)
    # X dup rows (parity-dependent)
    xz[:, 0::2, 2 * DZ + 2:128, BLK:2 * BLK] = xt[:, 0::2, 0:NDUP, :]
    xz[:, 1::2, 2 * DZ + 2:128, BLK:2 * BLK] = xt[:, 1::2, NDUP:2 * NDUP, :]

    # ---- weights ----------------------------------------------------------
    w1s = (S1 * W1).astype(np.float32)                  # [160, 256]
    w1x_hi = _q8(w1s[0:DX])                             # [128, 256]
    w1x_lo = _q8(2.0 * (w1s[0:DX] - w1x_hi.astype(np.float32)))
    w1z_hi = _q8(w1s[DX:DX + DZ])                       # [32, 256]
    # even blocks: Z-data-lo rows, scaled 2x (-> /8); odd: W1z quant error, 2x
    w1z_lo_even = (w1z_hi.astype(np.float32) / 8.0).astype(FP8)
    w1z_lo_odd = _q8(2.0 * (w1s[DX:DX + DZ] - w1z_hi.astype(np.float32)))
    b1s = (S1 * b1).astype(np.float32)
    b1hi = _q8(b1s)
    b1lo = _q8(b1s - b1hi.astype(np.float32))

    # stationary ktile rows [256, 256] then pack [parity][m][p, k*128+mc]
    w1q = np.zeros((2, 2, 128, 2 * 128), dtype=FP8)
    for par in range(2):
        st = np.zeros((256, H), dtype=FP8)
        st[0:128] = w1x_hi
        st[128 + 0:128 + DZ] = w1z_hi
        st[128 + DZ:128 + 2 * DZ] = w1z_lo_even if par == 0 else w1z_lo_odd
        st[128 + 2 * DZ] = b1hi
        st[128 + 2 * DZ + 1] = b1lo
        st[128 + 2 * DZ + 2:256] = w1x_lo[par * NDUP:(par + 1) * NDUP]
        for m in range(2):
            w1q[par, m] = (st.reshape(2, 128, H).transpose(1, 0, 2)
                           [:, :, m * 128:(m + 1) * 128].reshape(128, 256))

    w2s = (S2 * W2).astype(np.float32)
    t_hi = _q8(w2s)
    t_lo = _q8(2.0 * (w2s - t_hi.astype(np.float32)))
    w2q = np.zeros((2, 2, 128, 2 * 128), dtype=FP8)
    for t, term in enumerate((t_hi, t_lo)):
        for m in range(2):
            w2q[t, m] = (term.reshape(2, 128, H).transpose(1, 0, 2)
                         [:, :, m * 128:(m + 1) * 128].reshape(128, 256))
    b2d = np.ascontiguousarray((S1 * S2 * b2).reshape(2, 128, 1)) \
        .astype(np.float32)

    wq_host = np.concatenate(
        [w1q[0, 0], w1q[0, 1], w1q[1, 0], w1q[1, 1],
         w2q[0, 0], w2q[0, 1], w2q[1, 0], w2q[1, 1]], axis=1)

    # ---- run on 8 cores ---------------------------------------------------
    nc = _get_program(nblk)
    in_maps = []
    for c in range(N_CORES):
        in_maps.append({"xz": xz[c], "wq": wq_host, "b2": b2d})
    global _last_in_maps
    _last_in_maps = in_maps
    res = run_bass_kernel_spmd(nc, in_maps, list(range(N_CORES)))

    # ---- host epilogue ----------------------------------------------------
    per_core = []
    for c in range(N_CORES):
        o = res.results[c]["out"].reshape(128, nblk, 2, BLK // MC)
        per_core.append(np.concatenate(
            [o[:, :, 0, :].reshape(128, mc_per_core),
             o[:, :, 1, :].reshape(128, mc_per_core)], axis=0))
    sums = np.concatenate(per_core, axis=1)   # [256, n_mc], scaled S1*S2

    # analytic pad-cell contribution (X=0, Z=0, ones=1), parity-dependent
    pre1_pad = b1hi.astype(np.float32) + b1lo.astype(np.float32)  # [256]
    h1_pad = _q8(np.maximum(pre1_pad, 0.0)).astype(np.float32)
    w2hi_f = t_hi.astype(np.float32)
    w2lo_f = t_lo.astype(np.float32)
    b2s = (S1 * S2 * b2).astype(np.float32)
    v_pad_even = np.maximum(h1_pad @ (w2hi_f + w2lo_f) + b2s, 0.0) \
        .astype(BF16).astype(np.float32)
    v_pad_odd = np.maximum(h1_pad @ w2hi_f + b2s, 0.0) \
        .astype(BF16).astype(np.float32)
    blk_of_mc = (np.arange(n_mc) // (BLK // MC)) % nblk
    mc_parity = blk_of_mc % 2
    v_pad = np.where(mc_parity[None, :] == 0,
                     v_pad_even[:, None], v_pad_odd[:, None])
    sums = sums - v_pad * (MC - mc_real).astype(np.float32)[None, :]
    sums /= np.float32(S1 * S2)

    valid = mc_label >= 0
    S = np.zeros((nseg, H), dtype=np.float32)
    np.add.at(S, mc_label[valid], sums[:, valid].T)

    denom = np.maximum(counts, 1).astype(np.float32)[:, None]
    Y = S @ W3 / denom + b3[None, :]
    Y[counts == 0] = 0.0
    return Y.astype(np.float32)


BLK_ONES0 = 2 * DZ      # row 64 of ktile1 (ones -> b1 hi)
BLK_ONES1 = 2 * DZ + 1  # row 65 (ones -> b1 lo)
